# revision 8
# baseline (speedup 1.0000x reference)
"""Block floating-point quantization (shared-exponent, m-bit mantissa) on 8 trn2 cores.

out = clip(round(x / s), -2^(m-1), 2^(m-1)-1) * s,  s = 2^(floor(log2(blockmax)) - (m-1)),
blockmax = max |x| over each 16-element block along the last dim.

Implementation notes:
- Fully data-parallel: x (4,4096,4096) -> (16384,4096) row-sharded 8 ways; blocks are
  local to the last dim so shards are independent.
- Engine budget per [128, 8192] f32 tile (8 tiles per core), measured:
  * DMA: 16 engines x ~26 GB/s ~= 414 GB/s/core peak. f32-in/f32-out (8 B/elem) ran
    181 us at 92% engine busy; bf16 stores cut traffic to 6 B/elem (~125-132 us wall).
    Every quantized value is k * 2^(e-7), |k| <= 128 -> exactly representable in
    bf16; host astype(float32) restores f32 bit-for-bit.
  * DVE is then the bottleneck: windowed tensor_reduce never packs (8.7us/Melem at
    any dtype), and the fused quant op (6 ALU stages, f32 in) is a fixed 8.9us.
    The blockmax instead runs as a binary max tensor_tensor tree on contiguous bf16
    (2x_1P packed: 2.28/1.22/0.69/0.69 us) fed by the scalar engine.
  * ACT (otherwise idle) supplies the tree input: either Abs(x)->bf16 (fast path;
    RNE rounding can bump ~0.2% of block exponents, rel err ~2e-4, far under the
    2e-2 gate) or a bit-exact u16 high-half extract + DVE and-mask at 4x (exact
    path, +2.3us DVE/tile).
- Per tile, with m = blockmax (bf16) [P, nb]:
    (m16 & 0x7F80) + 0x0840 -> M = 1.5 * 2^16 * 2^e (e = shared exponent), the magic
    rounding constant, as bf16 (bf16->f32 read conversion in the DVE is exact).
    For an all-zero block m=0 gives M ~ 1e-34 and the output stays exactly 0.
    custom fused DVE op:  out_bf16 = (min(max(x, M*c0), M*c1) + M) - M
    c0 = -1/98304, c1 = (127/128)/98304, M broadcast per 16-elem block (stride-0 AP).
    The +M/-M pair rounds to the nearest multiple of 2^(e-7) with round-half-to-even
    (IEEE RNE on the fp32 adder), exactly matching jnp.round; the clip bounds are
    -128*s and 127*s up to 1ulp, which the subsequent rounding snaps to the exact
    grid values, so results match the f32 reference bit-for-bit (mod +/-0) in the
    exact path.
"""

import numpy as np

_MB = 8  # mantissa bits (incl. sign) this kernel is specialized for
_BS = 16  # block size

_prog_cache = {}
_op_cache = {}


def _get_custom_op(mb):
    """Register (once per process) the fused clip+round-to-grid DVE op."""
    if mb in _op_cache:
        return _op_cache[mb]
    from concourse import dve_ops
    from concourse.dve_ops import DveOp, OPS, _SUB_OPCODE_FOR_NAME, CUSTOM_DVE_SPECS
    from concourse.dve_spec import Spec, Src0, Src1, C0, C1, maxx, minn, lower, _has_src1
    from concourse.dve_uop import DveOpSpec

    name = f"BFP_QUANT_M{mb}_ANT"
    if name in _SUB_OPCODE_FOR_NAME:
        op = next(o for o in OPS if o.name == name)
        _op_cache[mb] = op
        return op

    def _ref(in0, in1, s0, s1, imm2):
        f32 = np.float32
        a = np.asarray(in0, f32)
        m = np.asarray(in1, f32).reshape(a.shape)
        lo = (m * f32(s0)).astype(f32)
        hi = (m * f32(s1)).astype(f32)
        t = np.minimum(np.maximum(a, lo), hi).astype(f32)
        return ((t + m).astype(f32) - m).astype(f32)

    body = (minn(maxx(Src0, Src1 * C0), Src1 * C1) + Src1) - Src1
    spec = Spec(body=body, reference=_ref)

    row = max(_SUB_OPCODE_FOR_NAME.values()) + 1
    assert row < 0x20, "custom-DVE opcode rows exhausted"
    _SUB_OPCODE_FOR_NAME[name] = row

    shas = {}
    for ver in ("v3", "v4"):
        tmp = DveOpSpec(
            name=name, opcode=row, uops=lower(spec, ver=ver), rd1_en=_has_src1(spec)
        )
        shas[ver] = tmp.sha(ver)

    op = DveOp(name, spec, subdim=False, uops_sha=shas)
    OPS.append(op)
    CUSTOM_DVE_SPECS[name] = spec
    _op_cache[mb] = op
    return op


def _build_program(rows, cols, bs, mb, bufs=3, split_ends=True, exact=True):
    """Build the single-core bass program (SPMD across all cores)."""
    key = (rows, cols, bs, mb, bufs, split_ends, exact)
    if key in _prog_cache:
        return _prog_cache[key]

    import concourse.bass as bass
    import concourse.tile as tile
    from concourse import mybir

    op = _get_custom_op(mb)

    mc = 1.5 * 2.0 ** (24 - mb)  # M / 2^e
    c0 = -1.0 / mc  # lo = -2^(mb-1) * s = -2^e
    c1 = (1.0 - 2.0 ** (1 - mb)) / mc  # hi = (2^(mb-1)-1) * s
    add_bits16 = ((24 - mb) << 7) | 0x40  # bf16-space M_bits - e_bits

    P = 128
    assert rows % P == 0 and cols % bs == 0
    ntiles = rows // P

    nc = bass.Bass()
    x_d = nc.declare_dram_parameter("x", [rows, cols], mybir.dt.float32, isOutput=False)
    o_d = nc.declare_dram_parameter(
        "out", [rows, cols], mybir.dt.bfloat16, isOutput=True
    )

    with tile.TileContext(nc) as tc:
        with (
            tc.tile_pool(name="xp", bufs=bufs) as xp,
            tc.tile_pool(name="ap", bufs=bufs) as abp,
            tc.tile_pool(name="op", bufs=bufs) as outp,
            tc.tile_pool(name="mp", bufs=bufs) as mp,
        ):
            def emit(r0, col0, w):
                nbw = w // bs
                xt = xp.tile([P, w], mybir.dt.float32)
                nc.sync.dma_start(xt[:], x_d[r0 : r0 + P, col0 : col0 + w])

                # |x| bits as contiguous bf16: the (otherwise idle) scalar
                # engine copies the high u16 of each f32 (bit-exact; Abs on
                # ACT is LUT-approximated and NOT usable), then the DVE
                # clears the sign bit in-place (single-src tensor_scalar
                # packs at 4x).
                at = abp.tile([P, w], mybir.dt.bfloat16)
                nc.scalar.activation(
                    out=at[:].bitcast(mybir.dt.uint16),
                    in_=xt[:].bitcast(mybir.dt.uint16)[:, 1::2],
                    func=mybir.ActivationFunctionType.Copy,
                )
                ai = at[:].bitcast(mybir.dt.int16)
                nc.vector.tensor_scalar(
                    out=ai, in0=ai, scalar1=0x7FFF, scalar2=None,
                    op0=mybir.AluOpType.bitwise_and,
                )

                # blockmax via a 4-level binary max tree, all 16-bit stride-1
                # (2x-packed tensor_tensor). Scratch lives inside the not-yet-
                # written out tile: all tree ops and the quant op run in order
                # on the DVE, so the quant write cannot race the tree reads.
                ot = outp.tile([P, w], mybir.dt.bfloat16)
                av = at[:].rearrange("p (b k) -> p b k", k=bs)
                t8 = ot[:, : nbw * 8].rearrange("p (b k) -> p b k", k=8)
                t4 = ot[:, nbw * 8 : nbw * 12].rearrange("p (b k) -> p b k", k=4)
                t2 = ot[:, nbw * 12 : nbw * 14].rearrange("p (b k) -> p b k", k=2)
                m = mp.tile([P, nbw], mybir.dt.bfloat16)
                nc.vector.tensor_tensor(
                    out=t8, in0=av[:, :, 0:8], in1=av[:, :, 8:16],
                    op=mybir.AluOpType.max,
                )
                nc.vector.tensor_tensor(
                    out=t4, in0=t8[:, :, 0:4], in1=t8[:, :, 4:8],
                    op=mybir.AluOpType.max,
                )
                nc.vector.tensor_tensor(
                    out=t2, in0=t4[:, :, 0:2], in1=t4[:, :, 2:4],
                    op=mybir.AluOpType.max,
                )
                nc.vector.tensor_tensor(
                    out=m[:].unsqueeze(2), in0=t2[:, :, 0:1], in1=t2[:, :, 1:2],
                    op=mybir.AluOpType.max,
                )
                mi = m[:].bitcast(mybir.dt.int16)
                nc.vector.tensor_scalar(
                    out=mi, in0=mi, scalar1=0x7F80, scalar2=None,
                    op0=mybir.AluOpType.bitwise_and,
                )
                nc.vector.tensor_scalar(
                    out=mi, in0=mi, scalar1=add_bits16, scalar2=None,
                    op0=mybir.AluOpType.add,
                )
                m_bcast = m[:].unsqueeze(2).broadcast_to([P, nbw, bs])
                nc.vector._custom_dve(
                    op, out=ot[:], in0=xt[:], in1=m_bcast, s0=c0, s1=c1
                )
                nc.sync.dma_start(o_d[r0 : r0 + P, col0 : col0 + w], ot[:])

            half = cols // 2
            for t in range(ntiles):
                # Optionally split the first and last tiles in half: shorter
                # pipeline ramp (first compute starts sooner) and tail (last
                # store is half the size), with full-size DMAs in between.
                if split_ends and t in (0, ntiles - 1) and half % bs == 0:
                    emit(t * P, 0, half)
                    emit(t * P, half, half)
                else:
                    emit(t * P, 0, cols)

    # Two post-passes the raw-Bass/Tile path doesn't run (Bacc.compile does):
    # - generate_event_semaphores: TRN2 allows at most 1 sync wait per
    #   instruction; splits excess waits into InstEventSemaphore.
    # - codegen_inst_isa_subclasses: populates .instr bytes for InstISA
    #   subclasses (InstCustomDveAnt); NEFF compile fails with "ISA wrong
    #   length" on empty .instr otherwise.
    from concourse.bass_utils import bass_rust

    bass_rust.generate_event_semaphores(nc)
    mybir.codegen_inst_isa_subclasses(nc)

    _prog_cache[key] = nc
    return nc


def _run(x2d, bs, mb, trace=False, cols=8192, bufs=3, split_ends=True, exact=True):
    """x2d: (R, C) float32, R % (8*128) == 0. Returns (out2d, BassKernelResults)."""
    from concourse.bass_utils import run_bass_kernel_spmd

    n_cores = 8
    R, C = x2d.shape
    per = R // n_cores
    if cols is None or (per * C) % (128 * cols) != 0:
        cols = C
    shard_rows = per * C // cols
    nc = _build_program(
        shard_rows, cols, bs, mb, bufs=bufs, split_ends=split_ends, exact=exact
    )

    in_maps = [
        {"x": np.ascontiguousarray(x2d[i * per : (i + 1) * per]).reshape(shard_rows, cols)}
        for i in range(n_cores)
    ]
    res = run_bass_kernel_spmd(nc, in_maps, list(range(n_cores)), trace=trace)
    out = np.empty_like(x2d)
    for i in range(n_cores):
        out[i * per : (i + 1) * per] = (
            res.results[i]["out"].astype(np.float32).reshape(per, C)
        )
    return out, res


def kernel(x, mantissa_bits=_MB, block_size=_BS):
    x = np.asarray(x, dtype=np.float32)
    mb = int(mantissa_bits)
    bs = int(block_size)
    shape = x.shape
    x2d = np.ascontiguousarray(x.reshape(-1, shape[-1]))
    out2d, _ = _run(x2d, bs, mb, trace=False)
    return out2d.reshape(shape)


# revision 13
# speedup vs baseline: 1.0113x; 1.0113x over previous
"""Block floating-point quantization (shared-exponent, m-bit mantissa) on 8 trn2 cores.

out = clip(round(x / s), -2^(m-1), 2^(m-1)-1) * s,  s = 2^(floor(log2(blockmax)) - (m-1)),
blockmax = max |x| over each 16-element block along the last dim.

Implementation notes:
- Fully data-parallel: x (4,4096,4096) -> (16384,4096) row-sharded 8 ways; blocks are
  local to the last dim so shards are independent.
- Engine budget per [128, 8192] f32 tile (8 tiles per core), measured:
  * DMA: 16 engines x ~26 GB/s ~= 414 GB/s/core peak. f32-in/f32-out (8 B/elem) ran
    181 us at 92% engine busy; bf16 stores cut traffic to 6 B/elem (~125-132 us wall).
    Every quantized value is k * 2^(e-7), |k| <= 128 -> exactly representable in
    bf16; host astype(float32) restores f32 bit-for-bit.
  * DVE is then the bottleneck: windowed tensor_reduce never packs (8.7us/Melem at
    any dtype), and the fused quant op (6 ALU stages, f32 in) is a fixed 8.9us.
    The blockmax instead runs as a binary max tensor_tensor tree on contiguous bf16
    (2x_1P packed: 2.28/1.22/0.69/0.69 us) fed by the scalar engine.
  * ACT (otherwise idle) supplies the tree input: either Abs(x)->bf16 (fast path;
    RNE rounding can bump ~0.2% of block exponents, rel err ~2e-4, far under the
    2e-2 gate) or a bit-exact u16 high-half extract + DVE and-mask at 4x (exact
    path, +2.3us DVE/tile).
- Per tile, with m = blockmax (bf16) [P, nb]:
    (m16 & 0x7F80) + 0x0840 -> M = 1.5 * 2^16 * 2^e (e = shared exponent), the magic
    rounding constant, as bf16 (bf16->f32 read conversion in the DVE is exact).
    For an all-zero block m=0 gives M ~ 1e-34 and the output stays exactly 0.
    custom fused DVE op:  out_bf16 = (min(max(x, M*c0), M*c1) + M) - M
    c0 = -1/98304, c1 = (127/128)/98304, M broadcast per 16-elem block (stride-0 AP).
    The +M/-M pair rounds to the nearest multiple of 2^(e-7) with round-half-to-even
    (IEEE RNE on the fp32 adder), exactly matching jnp.round; the clip bounds are
    -128*s and 127*s up to 1ulp, which the subsequent rounding snaps to the exact
    grid values, so results match the f32 reference bit-for-bit (mod +/-0) in the
    exact path.
"""

import numpy as np

_MB = 8  # mantissa bits (incl. sign) this kernel is specialized for
_BS = 16  # block size

_prog_cache = {}
_op_cache = {}


def _get_custom_op(mb):
    """Register (once per process) the fused clip+round-to-grid DVE op."""
    if mb in _op_cache:
        return _op_cache[mb]
    from concourse import dve_ops
    from concourse.dve_ops import DveOp, OPS, _SUB_OPCODE_FOR_NAME, CUSTOM_DVE_SPECS
    from concourse.dve_spec import Spec, Src0, Src1, C0, C1, maxx, minn, lower, _has_src1
    from concourse.dve_uop import DveOpSpec

    name = f"BFP_QUANT_M{mb}_ANT"
    if name in _SUB_OPCODE_FOR_NAME:
        op = next(o for o in OPS if o.name == name)
        _op_cache[mb] = op
        return op

    def _ref(in0, in1, s0, s1, imm2):
        f32 = np.float32
        a = np.asarray(in0, f32)
        m = np.asarray(in1, f32).reshape(a.shape)
        lo = (m * f32(s0)).astype(f32)
        hi = (m * f32(s1)).astype(f32)
        t = np.minimum(np.maximum(a, lo), hi).astype(f32)
        return ((t + m).astype(f32) - m).astype(f32)

    body = (minn(maxx(Src0, Src1 * C0), Src1 * C1) + Src1) - Src1
    spec = Spec(body=body, reference=_ref)

    row = max(_SUB_OPCODE_FOR_NAME.values()) + 1
    assert row < 0x20, "custom-DVE opcode rows exhausted"
    _SUB_OPCODE_FOR_NAME[name] = row

    shas = {}
    for ver in ("v3", "v4"):
        tmp = DveOpSpec(
            name=name, opcode=row, uops=lower(spec, ver=ver), rd1_en=_has_src1(spec)
        )
        shas[ver] = tmp.sha(ver)

    op = DveOp(name, spec, subdim=False, uops_sha=shas)
    OPS.append(op)
    CUSTOM_DVE_SPECS[name] = spec
    _op_cache[mb] = op
    return op


def _build_program(rows, cols, bs, mb, bufs=3, split_ends=True, madd_on_act=False):
    """Build the single-core bass program (SPMD across all cores)."""
    key = (rows, cols, bs, mb, bufs, split_ends, madd_on_act)
    if key in _prog_cache:
        return _prog_cache[key]

    import concourse.bass as bass
    import concourse.tile as tile
    from concourse import mybir

    op = _get_custom_op(mb)

    mc = 1.5 * 2.0 ** (24 - mb)  # M / 2^e
    c0 = -1.0 / mc  # lo = -2^(mb-1) * s = -2^e
    c1 = (1.0 - 2.0 ** (1 - mb)) / mc  # hi = (2^(mb-1)-1) * s
    add_bits16 = ((24 - mb) << 7) | 0x40  # bf16-space M_bits - e_bits

    P = 128
    assert rows % P == 0 and cols % bs == 0
    ntiles = rows // P

    nc = bass.Bass()
    x_d = nc.declare_dram_parameter("x", [rows, cols], mybir.dt.float32, isOutput=False)
    o_d = nc.declare_dram_parameter(
        "out", [rows, cols], mybir.dt.bfloat16, isOutput=True
    )

    if isinstance(bufs, int):
        bx = ba = bo = bm = bufs
    else:
        bx, ba, bo, bm = bufs
    with tile.TileContext(nc) as tc:
        with (
            tc.tile_pool(name="xp", bufs=bx) as xp,
            tc.tile_pool(name="ap", bufs=ba) as abp,
            tc.tile_pool(name="op", bufs=bo) as outp,
            tc.tile_pool(name="mp", bufs=bm) as mp,
        ):
            def emit(r0, col0, w):
                nbw = w // bs
                xt = xp.tile([P, w], mybir.dt.float32)
                nc.sync.dma_start(xt[:], x_d[r0 : r0 + P, col0 : col0 + w])

                # |x| bits as contiguous bf16: the (otherwise idle) scalar
                # engine copies the high u16 of each f32 (bit-exact; Abs on
                # ACT is LUT-approximated and NOT usable), then the DVE
                # clears the sign bit in-place (single-src tensor_scalar
                # packs at 4x).
                at = abp.tile([P, w], mybir.dt.bfloat16)
                nc.scalar.activation(
                    out=at[:].bitcast(mybir.dt.uint16),
                    in_=xt[:].bitcast(mybir.dt.uint16)[:, 1::2],
                    func=mybir.ActivationFunctionType.Copy,
                )
                # Clear sign AND mantissa in one 4x-packed pass: the shared
                # exponent is max over per-element exponents, so the tree can
                # run on pure 2^e values.
                ai = at[:].bitcast(mybir.dt.int16)
                nc.vector.tensor_scalar(
                    out=ai, in0=ai, scalar1=0x7F80, scalar2=None,
                    op0=mybir.AluOpType.bitwise_and,
                )

                # blockmax via a 4-level binary max tree, all 16-bit stride-1
                # (2x-packed tensor_tensor). Scratch lives inside the not-yet-
                # written out tile: all tree ops and the quant op run in order
                # on the DVE, so the quant write cannot race the tree reads.
                ot = outp.tile([P, w], mybir.dt.bfloat16)
                av = at[:].rearrange("p (b k) -> p b k", k=bs)
                t8 = ot[:, : nbw * 8].rearrange("p (b k) -> p b k", k=8)
                t4 = ot[:, nbw * 8 : nbw * 12].rearrange("p (b k) -> p b k", k=4)
                t2 = ot[:, nbw * 12 : nbw * 14].rearrange("p (b k) -> p b k", k=2)
                m = mp.tile([P, nbw], mybir.dt.bfloat16)
                nc.vector.tensor_tensor(
                    out=t8, in0=av[:, :, 0:8], in1=av[:, :, 8:16],
                    op=mybir.AluOpType.max,
                )
                nc.vector.tensor_tensor(
                    out=t4, in0=t8[:, :, 0:4], in1=t8[:, :, 4:8],
                    op=mybir.AluOpType.max,
                )
                nc.vector.tensor_tensor(
                    out=t2, in0=t4[:, :, 0:2], in1=t4[:, :, 2:4],
                    op=mybir.AluOpType.max,
                )
                nc.vector.tensor_tensor(
                    out=m[:].unsqueeze(2), in0=t2[:, :, 0:1], in1=t2[:, :, 1:2],
                    op=mybir.AluOpType.max,
                )
                # m bits are already (e+127)<<7; build M = m_bits + 0x0840 on
                # the scalar engine (Copy with float bias is exact on int16
                # values), keeping the DVE free.
                mi = m[:].bitcast(mybir.dt.int16)
                if madd_on_act:
                    nc.scalar.activation(
                        out=mi, in_=mi,
                        func=mybir.ActivationFunctionType.Copy,
                        bias=float(add_bits16),
                    )
                else:
                    nc.vector.tensor_scalar(
                        out=mi, in0=mi, scalar1=add_bits16, scalar2=None,
                        op0=mybir.AluOpType.add,
                    )
                m_bcast = m[:].unsqueeze(2).broadcast_to([P, nbw, bs])
                nc.vector._custom_dve(
                    op, out=ot[:], in0=xt[:], in1=m_bcast, s0=c0, s1=c1
                )
                nc.sync.dma_start(o_d[r0 : r0 + P, col0 : col0 + w], ot[:])

            half = cols // 2
            for t in range(ntiles):
                # Split the first and last tiles in half: shorter pipeline
                # ramp (first compute starts sooner) and tail (last store is
                # half the size), with full-size DMAs in between.
                if split_ends and t in (0, ntiles - 1) and half % bs == 0:
                    emit(t * P, 0, half)
                    emit(t * P, half, half)
                else:
                    emit(t * P, 0, cols)

    # Two post-passes the raw-Bass/Tile path doesn't run (Bacc.compile does):
    # - generate_event_semaphores: TRN2 allows at most 1 sync wait per
    #   instruction; splits excess waits into InstEventSemaphore.
    # - codegen_inst_isa_subclasses: populates .instr bytes for InstISA
    #   subclasses (InstCustomDveAnt); NEFF compile fails with "ISA wrong
    #   length" on empty .instr otherwise.
    from concourse.bass_utils import bass_rust

    bass_rust.generate_event_semaphores(nc)
    mybir.codegen_inst_isa_subclasses(nc)

    _prog_cache[key] = nc
    return nc


def _run(x2d, bs, mb, trace=False, cols=8192, bufs=3, split_ends=True, madd_on_act=False):
    """x2d: (R, C) float32, R % (8*128) == 0. Returns (out2d, BassKernelResults)."""
    from concourse.bass_utils import run_bass_kernel_spmd

    n_cores = 8
    R, C = x2d.shape
    per = R // n_cores
    if cols is None or (per * C) % (128 * cols) != 0:
        cols = C
    shard_rows = per * C // cols
    nc = _build_program(
        shard_rows, cols, bs, mb, bufs=bufs, split_ends=split_ends,
        madd_on_act=madd_on_act,
    )

    in_maps = [
        {"x": np.ascontiguousarray(x2d[i * per : (i + 1) * per]).reshape(shard_rows, cols)}
        for i in range(n_cores)
    ]
    res = run_bass_kernel_spmd(nc, in_maps, list(range(n_cores)), trace=trace)
    out = np.empty_like(x2d)
    for i in range(n_cores):
        out[i * per : (i + 1) * per] = (
            res.results[i]["out"].astype(np.float32).reshape(per, C)
        )
    return out, res


def kernel(x, mantissa_bits=_MB, block_size=_BS):
    x = np.asarray(x, dtype=np.float32)
    mb = int(mantissa_bits)
    bs = int(block_size)
    shape = x.shape
    x2d = np.ascontiguousarray(x.reshape(-1, shape[-1]))
    out2d, _ = _run(x2d, bs, mb, trace=False)
    return out2d.reshape(shape)
